# revision 4
# baseline (speedup 1.0000x reference)
"""Trainium2 Bass kernel for the CapibaraByte recurrent-scan problem.

Reference computation (B=128, T=1024, D_IN=256, H=2048):
    conv = einsum('btd,dh->bth', x, W_conv)
    step:  s <- 0.9*s + 0.1*gelu(s @ W_state + conv[:,t] + bias)
    out = (s @ W_state + bias, s)

Strategy: data-parallel over batch across 8 cores (B_local=16/core); the
scan runs fully on-core with zero cross-core traffic.  The per-step GEMM
(16 x 2048) @ (2048 x 2048) is done state-stationary (state as the PE
weights) with 4-way column tiling so four 16-wide weight tiles stream
concurrently on disjoint PE column groups.  The state lives in transposed
[h, b] layout; each step the packed [b, h] matmul output is transposed
back with 16 PE transpose ops.  All matmul operands are bf16 (fp32 PSUM
accumulate); a fp32 master copy of the state keeps the blend exact.  The
x @ W_conv projection is fused into the loop in T-blocks so conv features
never round-trip DRAM.
"""

import sys

for _p in ("/opt/trn_rl_repo",):
    if _p not in sys.path:
        sys.path.insert(0, _p)

import numpy as np
import ml_dtypes

import concourse.bass as bass
import concourse.tile as tile
from concourse import bacc, mybir
from concourse.bass import ds
from concourse.bass_utils import run_bass_kernel_spmd

AFT = mybir.ActivationFunctionType
ALU = mybir.AluOpType
F32 = mybir.dt.float32
BF16 = mybir.dt.bfloat16

B, T_FULL, D_IN, H = 128, 1024, 256, 2048
NCORES = 8
BL = B // NCORES            # 16 batch rows per core
KT = H // 128               # 16 contraction tiles
MT = H // 128               # 16 output h-tiles
UPDATE = 0.1


def build(T_steps=T_FULL, U=8, act=AFT.Gelu_apprx_tanh, n_repeat=1):
    """Build the Bacc graph for a T_steps-long scan, U steps per loop iter."""
    assert T_steps % U == 0
    nc = bacc.Bacc("TRN2", target_bir_lowering=False, debug=False,
                   num_devices=NCORES)

    xT_d = nc.dram_tensor("xT", [2, 128, T_steps * BL], BF16,
                          kind="ExternalInput").ap()
    w_d = nc.dram_tensor("w_arr", [128, KT * H], BF16,
                         kind="ExternalInput").ap()
    wc_d = nc.dram_tensor("wc_arr", [128, 2 * H], BF16,
                          kind="ExternalInput").ap()
    biasA_d = nc.dram_tensor("bias_arr", [128, MT], F32,
                             kind="ExternalInput").ap()
    biasT_d = nc.dram_tensor("bias_bcT", [128, MT * BL], F32,
                             kind="ExternalInput").ap()
    ident_d = nc.dram_tensor("ident", [128, 128], F32,
                             kind="ExternalInput").ap()
    outT_d = nc.dram_tensor("outT", [128, MT * BL], F32,
                            kind="ExternalOutput").ap()
    stT_d = nc.dram_tensor("stT", [128, MT * BL], F32,
                           kind="ExternalOutput").ap()

    UB = U * BL  # conv block column count per k-tile

    with tile.TileContext(nc) as tc:
        with (
            tc.tile_pool(name="persist", bufs=1) as persist,
            tc.tile_pool(name="xin", bufs=2) as xpool,
            tc.tile_pool(name="cblk", bufs=2) as cpool,
            tc.tile_pool(name="work", bufs=2) as work,
            tc.tile_pool(name="psum_su", bufs=1, space="PSUM") as psum_su,
            tc.tile_pool(name="psum_t", bufs=2, space="PSUM") as psum_t,
            tc.tile_pool(name="psum_c", bufs=2, space="PSUM") as psum_c,
        ):
            # ---- resident tensors ----
            w_sb = persist.tile([128, KT * H], BF16, tag="w_sb")
            nc.sync.dma_start(w_sb[:], w_d[:])
            wc_sb = persist.tile([128, 2 * H], BF16, tag="wc_sb")
            nc.sync.dma_start(wc_sb[:], wc_d[:])
            bias_sb = persist.tile([128, MT], F32, tag="bias_sb")
            nc.sync.dma_start(bias_sb[:], biasA_d[:])
            biasT_sb = persist.tile([128, MT * BL], F32, tag="biasT_sb")
            nc.sync.dma_start(biasT_sb[:], biasT_d[:])
            ident_sb = persist.tile([128, 128], F32, tag="ident_sb")
            nc.sync.dma_start(ident_sb[:], ident_d[:])

            # state in [h, b] layout: col tau*BL+b, partition p -> h=128*tau+p
            stT_bf = persist.tile([128, MT * BL], BF16, tag="stT_bf")
            nc.vector.memset(stT_bf[:], 0.0)
            stT_f32 = persist.tile([128, MT * BL], F32, tag="stT_f32")
            nc.vector.memset(stT_f32[:], 0.0)

            def mm_phase():
                """64 col-tiled matmuls: packed su[32g+b, n] for n-chunk g."""
                sus = [psum_su.tile([128, 512], F32, tag=f"su{g}", name=f"su{g}")
                       for g in range(4)]
                for k in range(KT):
                    lhs = stT_bf[:, BL * k:BL * (k + 1)]
                    for g in range(4):
                        nc.tensor.matmul(
                            sus[g][32 * g:32 * g + BL, :],
                            lhsT=lhs,
                            rhs=w_sb[:, k * H + 512 * g:k * H + 512 * (g + 1)],
                            start=(k == 0), stop=(k == KT - 1),
                            tile_position=(0, 32 * g),
                        )
                return sus

            def evict_transpose(sus):
                """psum (packed [b,h]) -> base-0 sbuf f32 -> PE-transpose to [h,b].

                Transpose inputs must sit at partition base 0 (walrus rejects
                offset-partition transpose), so the evictions shift partitions
                32g -> 0 while copying.
                """
                su16 = work.tile([BL, H], F32, tag="su16")
                for g in range(4):
                    src = sus[g][32 * g:32 * g + BL, :]
                    dst = su16[:, 512 * g:512 * (g + 1)]
                    if g % 2 == 0:
                        nc.vector.tensor_copy(dst, src)
                    else:
                        nc.scalar.copy(dst, src)
                pT = psum_t.tile([128, MT * BL], F32, tag="pT")
                for tau in range(MT):
                    nc.tensor.matmul(
                        pT[:, BL * tau:BL * (tau + 1)],
                        lhsT=su16[:, 128 * tau:128 * (tau + 1)],
                        rhs=ident_sb[0:BL, 0:BL],
                        is_transpose=True, start=True, stop=True,
                    )
                return pT

            def do_step(cT, u):
                sus = mm_phase()
                pT = evict_transpose(sus)
                # su_T + (c_t + bias)   [c was biased at conv eviction]
                s1 = work.tile([128, MT * BL], F32, tag="s1")
                nc.vector.tensor_tensor(
                    s1[:], pT[:], cT[:, u * (MT * BL):(u + 1) * (MT * BL)],
                    ALU.add)
                gsb = work.tile([128, MT * BL], F32, tag="gsb")
                nc.scalar.activation(gsb[:], s1[:], act)
                tmp = work.tile([128, MT * BL], F32, tag="tmp")
                nc.vector.tensor_scalar_mul(tmp[:], stT_f32[:], 1.0 - UPDATE)
                nc.vector.scalar_tensor_tensor(
                    stT_f32[:], gsb[:], UPDATE, tmp[:], ALU.mult, ALU.add)
                nc.vector.tensor_copy(stT_bf[:], stT_f32[:])

            def conv_block(i):
                """c[tau*U..][p][u,b] for U steps, bias folded in, f32."""
                xblk = xpool.tile([128, 2 * UB], BF16, tag="xblk")
                for kc in range(2):
                    nc.sync.dma_start(
                        xblk[:, kc * UB:(kc + 1) * UB],
                        xT_d[kc, :, ds(i * UB, UB)])
                cT = cpool.tile([128, U * MT * BL], F32, tag="cT")
                cT_r = cT[:].rearrange("p (u z) -> p u z", z=MT * BL)
                for m in range(MT):
                    pc = psum_c.tile([128, UB], F32, tag="pc")
                    for kc in range(2):
                        nc.tensor.matmul(
                            pc[:],
                            lhsT=wc_sb[:, kc * H + 128 * m:kc * H + 128 * (m + 1)],
                            rhs=xblk[:, kc * UB:(kc + 1) * UB],
                            start=(kc == 0), stop=(kc == 1))
                    # pc free dim is (u, b); scatter to cT cols u*256 + m*16 + b
                    nc.scalar.activation(
                        cT_r[:, :, BL * m:BL * (m + 1)], pc[:],
                        AFT.Identity, bias=bias_sb[:, m:m + 1])
                return cT

            n_iters = T_steps // U

            def main_loop():
                with tc.For_i(0, n_iters, 1,
                              hint_engines=(mybir.EngineType.PE,)) as i:
                    cT = conv_block(i)
                    for u in range(U):
                        do_step(cT, u)

            if n_repeat == 1:
                main_loop()
            else:
                with tc.For_i(0, n_repeat, 1) as _r:
                    main_loop()

            # ---- final output = state @ W_state + bias ----
            sus = mm_phase()
            pT = evict_transpose(sus)
            outf = work.tile([128, MT * BL], F32, tag="outf")
            nc.vector.tensor_tensor(outf[:], pT[:], biasT_sb[:], ALU.add)
            nc.sync.dma_start(outT_d[:], outf[:])
            nc.sync.dma_start(stT_d[:], stT_f32[:])

    nc.compile()
    return nc


def host_inputs(x, W_state, W_conv, bias, T_steps=T_FULL):
    """Per-core input dicts. x: (B, T_steps, D_IN) f32."""
    bf = ml_dtypes.bfloat16
    # W_state [H, H] -> [128, k*H] with w[p, k*H+n] = W[128k+p, n]
    w_arr = np.ascontiguousarray(
        W_state.reshape(KT, 128, H).transpose(1, 0, 2).reshape(128, KT * H)
    ).astype(bf)
    wc_arr = np.ascontiguousarray(
        W_conv.reshape(2, 128, H).transpose(1, 0, 2).reshape(128, 2 * H)
    ).astype(bf)
    bias_arr = np.ascontiguousarray(bias.reshape(MT, 128).T).astype(np.float32)
    bias_bcT = np.repeat(bias.reshape(MT, 128).T[:, :, None], BL,
                         axis=2).reshape(128, MT * BL).astype(np.float32)
    ident = np.eye(128, dtype=np.float32)

    in_maps = []
    for c in range(NCORES):
        xs = x[c * BL:(c + 1) * BL]          # [BL, T, D]
        # xT[kc, p, t*BL+b] = xs[b, t, kc*128+p]
        xT = np.ascontiguousarray(
            xs.reshape(BL, T_steps, 2, 128).transpose(2, 3, 1, 0)
            .reshape(2, 128, T_steps * BL)).astype(bf)
        in_maps.append({
            "xT": xT, "w_arr": w_arr, "wc_arr": wc_arr,
            "bias_arr": bias_arr, "bias_bcT": bias_bcT, "ident": ident,
        })
    return in_maps


def gather_outputs(results):
    """results: list of per-core dicts -> (output, state) full arrays."""
    out = np.empty((B, H), np.float32)
    st = np.empty((B, H), np.float32)
    for c, r in enumerate(results):
        # arr[p, tau*BL+b] = val[b, 128*tau+p]
        o = r["outT"].reshape(128, MT, BL).transpose(2, 1, 0).reshape(BL, H)
        s = r["stT"].reshape(128, MT, BL).transpose(2, 1, 0).reshape(BL, H)
        out[c * BL:(c + 1) * BL] = o
        st[c * BL:(c + 1) * BL] = s
    return out, st


_NC_CACHE = {}


def _get_nc(T_steps=T_FULL, U=8, n_repeat=1):
    key = (T_steps, U, n_repeat)
    if key not in _NC_CACHE:
        _NC_CACHE[key] = build(T_steps, U, n_repeat=n_repeat)
    return _NC_CACHE[key]


def kernel(x, W_state, W_conv, bias):
    x = np.asarray(x, np.float32)
    W_state = np.asarray(W_state, np.float32)
    W_conv = np.asarray(W_conv, np.float32)
    bias = np.asarray(bias, np.float32)
    nc = _get_nc()
    in_maps = host_inputs(x, W_state, W_conv, bias)
    res = run_bass_kernel_spmd(nc, in_maps, list(range(NCORES)))
    return gather_outputs(res.results)

